# revision 1
# baseline (speedup 1.0000x reference)
"""Trainium2 Bass kernel for nn_Embedding2Score (segment_reduce).

Reference computation:
    v_n  = x[last_idx]                               [B, H]
    h    = sigmoid((v_n @ W1^T + b1)[batch] + x @ W2^T + b2)
    alpha= h @ q^T + q_b                             [N, 1]
    s_g  = segment_sum(alpha * x, batch)             [B, H]
    s_h  = [v_n, s_g] @ W3^T + b3                    [B, H]
    z    = s_h @ emb[1:]^T                           [B, V-1]

Sharding (8 cores): phase 1 is data-parallel over segments (256 sorted
sessions' worth of nodes per core); phase 2 is data-parallel over vocab
columns (12500 emb rows per core, all 2048 segments), with s_h
all-gathered on-device between the phases (merged single launch).

The rel-err budget (2e-2) is spent on bf16 throughout: phase 1 runs in
a transposed layout (stationary W2/W1-bias, bf16 moving operands at
1 cyc/row), with host-precomputed segment one-hot masks so no on-chip
transposes are needed; phase 2 is a single bf16 matmul per tile
(s_h and emb rounded to bf16, fp32 PSUM accumulate) and z is written
to DRAM as bf16 (halving the dominant ~100MB/core output traffic),
upcast to f32 on the host.
"""
import numpy as np
import ml_dtypes

import concourse.bass as bass
import concourse.tile as tile
import concourse.mybir as mybir
from concourse import bacc
from concourse import bass_utils
from concourse.masks import make_identity

F32 = mybir.dt.float32
BF16 = mybir.dt.bfloat16
FP8 = mybir.dt.float8e4
I32 = mybir.dt.int32
NPBF = ml_dtypes.bfloat16
NPF8 = ml_dtypes.float8_e4m3

N_NODES = 102400
B_SEG = 2048
H = 128
VOCAB = 100000
NCORES = 8
SEG_C = B_SEG // NCORES          # 256 segments per core
VSHARD = 12500                   # vocab columns per core (8*12500 covers 99999)
CW = 2048                        # phase-2 vocab chunk width
MW = 128                         # bias-path mask window (64-grid aligned)
MGRID = 64
SW = 40                          # s_g-path mask window (unquantized)


def affine_windows(nmax, mask_w, grid=1):
    """Core-uniform per-chunk segment-window starts (affine in chunk idx)."""
    nt = nmax // 128
    return [min(max(0, grid * round((round(n * SEG_C / nt) - mask_w // 2) / grid)),
                SEG_C - mask_w) for n in range(nt)]


# ---------------------------------------------------------------------------
# phase 1 (windowed): transposed-h layout, bf16
# ---------------------------------------------------------------------------

def _p1_declare(nc, nmax):
    nt = nmax // 128
    d = {}
    d["x"] = nc.dram_tensor("x", [nmax, H], BF16, kind="ExternalInput")
    d["xT"] = nc.dram_tensor("xT", [H, nmax], BF16, kind="ExternalInput")
    d["xph"] = nc.dram_tensor("xph", [128, nt * H], BF16, kind="ExternalInput")
    d["segmask"] = nc.dram_tensor("segmask", [128, nmax], FP8,
                                  kind="ExternalInput")
    d["nodemask"] = nc.dram_tensor("nodemask", [128, nt * SW], FP8,
                                   kind="ExternalInput")
    d["lastloc"] = nc.dram_tensor("lastloc", [128, 2], I32, kind="ExternalInput")
    # packed params: pbf = [W2T | W1T | W3aT | W3bT | qcol-pad] bf16,
    # pf32 = [b12 | w3brow | qb-pad] f32 — two DMAs instead of nine
    d["pbf"] = nc.dram_tensor("pbf", [H, 5 * H], BF16, kind="ExternalInput")
    d["pf32"] = nc.dram_tensor("pf32", [1, 3 * H], F32, kind="ExternalInput")
    return d


def _p1_body(nc, tc, d, nmax, windows, swin, pools):
    """Emit phase-1 IR. Returns shs tile [128, 2, H] f32 holding s_h."""
    nt = nmax // 128
    nsup = nmax // 512
    const, work, wk2, big, sgp = pools

    ident_bf = const.tile([128, 128], BF16)
    make_identity(nc, ident_bf[:])
    ones1 = const.tile([1, 128], F32)
    nc.vector.memset(ones1[:], 1.0)
    zrow = const.tile([1, SEG_C], F32)
    nc.vector.memset(zrow[:], 0.0)

    pbf = const.tile([H, 5 * H], BF16)
    nc.sync.dma_start(pbf[:], d["pbf"][:, :])
    pf32 = const.tile([1, 3 * H], F32)
    nc.sync.dma_start(pf32[:], d["pf32"][:, :])
    w2t = pbf[:, 0 * H:1 * H]
    w1t = pbf[:, 1 * H:2 * H]
    w3at = pbf[:, 2 * H:3 * H]
    w3bt = pbf[:, 3 * H:4 * H]
    qcol = pbf[:, 4 * H:4 * H + 1]
    b12 = pf32[:, 0 * H:1 * H]
    w3brow = pf32[:, 1 * H:2 * H]
    qb = pf32[:, 2 * H:2 * H + 1]
    lastloc = const.tile([128, 2], I32)
    nc.sync.dma_start(lastloc[:], d["lastloc"][:, :])

    # v_n gather + transpose + w1b2 FIRST: every bias matmul depends on
    # these tiny transfers, so they must not queue behind the bulk loads
    vn = const.tile([128, 2, H], BF16)
    vnT = const.tile([H, SEG_C], BF16)
    for t in range(2):
        nc.gpsimd.indirect_dma_start(
            out=vn[:, t, :], out_offset=None, in_=d["x"][:, :],
            in_offset=bass.IndirectOffsetOnAxis(ap=lastloc[:, t:t + 1], axis=0))
        tp = big.tile([128, 128], BF16, tag="bigp")
        nc.tensor.transpose(tp[:], vn[:, t, :], ident_bf[:])
        nc.vector.tensor_copy(vnT[:, t * 128:(t + 1) * 128], tp[:])

    # w1b2 blocks: one [MW, H] block per distinct window start
    blocks = sorted(set(windows))
    blkmap = {s0: t for t, s0 in enumerate(blocks)}
    w1b2 = const.tile([MW, len(blocks), H], BF16)
    for t, s0 in enumerate(blocks):
        pw = big.tile([128, 128], F32, tag="bigp")
        nc.tensor.matmul(pw[:MW, :], ones1[:, :MW], b12, start=True,
                         stop=False, skip_group_check=True)
        nc.tensor.matmul(pw[:MW, :], vnT[:, s0:s0 + MW], w1t,
                         start=False, stop=True, skip_group_check=True)
        nc.scalar.copy(w1b2[:, t, :], pw[:MW, :])

    # big streaming inputs, quartered so early supers can start sooner
    xT_sb = const.tile([H, nmax], BF16)
    segmask = const.tile([128, nmax], FP8)
    xph = const.tile([128, nt * H], BF16)
    nodemask = const.tile([128, nt * SW], FP8)
    qb_ = [(nmax // 4096) * 512 * i for i in range(8)] + [nmax]
    nm_loaded = False
    for q in range(8):
        s = slice(qb_[q], qb_[q + 1])
        if s.start >= s.stop:
            continue
        nc.sync.dma_start(xT_sb[:, s], d["xT"][:, s])
        nc.sync.dma_start(segmask[:, s], d["segmask"][:, s])
        if q >= 1 and not nm_loaded:
            nc.sync.dma_start(nodemask[:], d["nodemask"][:, :])
            nm_loaded = True
        if q % 2 == 1:
            sp = slice((qb_[q - 1] // 128) * H, (qb_[q + 1] // 128) * H)
            if sp.start < sp.stop:
                nc.sync.dma_start(xph[:, sp], d["xph"][:, sp])
    if not nm_loaded:
        nc.sync.dma_start(nodemask[:], d["nodemask"][:, :])

    # s_g accumulator (zero-init via PE)
    sg_ps = sgp.tile([128, SEG_C], F32, tag="sg")
    nc.tensor.matmul(sg_ps[:], ones1[:], zrow[:], start=True, stop=True,
                     skip_group_check=True)

    # software-pipelined emission: PE never waits on the ACT/DVE roundtrips
    # (alpha consumes hT two supers late, s_g consumes am two more late)
    L1, L2 = 3, 7
    hts, ams = {}, {}
    for it in range(nsup + L2):
        if it < nsup:
            g = it
            p1 = big.tile([128, 512], F32, tag="bigp")
            ns = slice(g * 512, (g + 1) * 512)
            nc.tensor.matmul(p1[:], w2t, xT_sb[:, ns],
                             start=True, stop=False, skip_group_check=True)
            # bias runs: consecutive chunks sharing a window block
            runs = []
            for c in range(4):
                n = g * 4 + c
                blk = blkmap[windows[n]]
                if runs and runs[-1][2] == blk:
                    runs[-1][1] = c + 1
                else:
                    runs.append([c, c + 1, blk])
            for ri, (c0, c1, blk) in enumerate(runs):
                nc.tensor.matmul(
                    p1[:, c0 * 128:c1 * 128], w1b2[:, blk, :],
                    segmask[:, g * 512 + c0 * 128:g * 512 + c1 * 128],
                    start=False, stop=(ri == len(runs) - 1),
                    skip_group_check=True)
            hT = work.tile([128, 512], BF16, tag="hT", bufs=L1 + 2)
            nc.scalar.activation(hT[:], p1[:],
                                 mybir.ActivationFunctionType.Sigmoid)
            hts[g] = hT
        if L1 <= it < nsup + L1:
            g = it - L1
            hT = hts.pop(g)
            aps = big.tile([128, 4], F32, tag="bigp")
            nc.tensor.matmul(aps[:], ones1[:], qb.to_broadcast([1, 4]),
                             start=True, stop=False, skip_group_check=True)
            for c in range(4):
                nc.tensor.matmul(aps[:, c:c + 1], hT[:, c * 128:(c + 1) * 128],
                                 qcol, start=False, stop=True,
                                 skip_group_check=True)
            am = wk2.tile([128, 4, SW], BF16, tag="am", bufs=L2 - L1 + 2)
            nc.vector.tensor_tensor(
                am[:], nodemask[:, g * 4 * SW:(g + 1) * 4 * SW].rearrange(
                    "p (c w) -> p c w", c=4),
                _bc(aps[:], 2, SW), op=mybir.AluOpType.mult)
            ams[g] = am
        if L2 <= it:
            g = it - L2
            am = ams.pop(g)
            for c in range(4):
                n = g * 4 + c
                nc.tensor.matmul(sg_ps[:, swin[n]:swin[n] + SW],
                                 xph[:, n * H:(n + 1) * H], am[:, c, :],
                                 start=False, stop=(n == nt - 1),
                                 skip_group_check=True)

    sgT = const.tile([128, SEG_C], BF16)
    nc.vector.tensor_copy(sgT[:], sg_ps[:])
    shs = const.tile([128, 2, H], F32)
    for t in range(2):
        psh = big.tile([128, 128], F32, tag="bigp")
        nc.tensor.matmul(psh[:], ones1[:], w3brow, start=True, stop=False,
                         skip_group_check=True)
        nc.tensor.matmul(psh[:], vnT[:, t * 128:(t + 1) * 128], w3at,
                         start=False, stop=False, skip_group_check=True)
        nc.tensor.matmul(psh[:], sgT[:, t * 128:(t + 1) * 128], w3bt,
                         start=False, stop=True, skip_group_check=True)
        nc.vector.tensor_copy(shs[:, t, :], psh[:])
    return shs


# ---------------------------------------------------------------------------
# phase 2: z = s_h @ emb_shard^T, single bf16 matmul, bf16 output
# ---------------------------------------------------------------------------

def _p2_chunks():
    out, off = [], 0
    while off < VSHARD:
        cw = min(CW, VSHARD - off)
        out.append((off, cw))
        off += cw
    return out


def _p2_body(nc, tc, shT, et_tiles, z_d, big, stage, tilew=512):
    k = 0
    half = 3 * CW
    for m in range(B_SEG // 128):
        stg = stage.tile([128, VSHARD], BF16, tag="stg")
        for i, (off, cw) in enumerate(_p2_chunks()):
            for h0 in range(0, cw, tilew):
                hw_ = min(tilew, cw - h0)
                pz = big.tile([128, tilew], F32, tag="bigp")
                for j0 in range(0, hw_, 512):
                    jw = min(512, hw_ - j0)
                    nc.tensor.matmul(
                        pz[:, j0:j0 + jw], shT[:, m * 128:(m + 1) * 128],
                        et_tiles[i][:, h0 + j0:h0 + j0 + jw],
                        start=True, stop=True, skip_group_check=True)
                dst = stg[:, off + h0:off + h0 + hw_]
                if k % 2 == 0:
                    nc.vector.tensor_copy(dst, pz[:, :hw_])
                else:
                    nc.scalar.copy(dst, pz[:, :hw_])
                k += 1
            if m == 0 and off + cw == half:
                nc.sync.dma_start(z_d[0:128, 0:half], stg[:, 0:half])
        if m == 0:
            nc.sync.dma_start(z_d[0:128, half:], stg[:, half:])
        else:
            nc.sync.dma_start(z_d[m * 128:(m + 1) * 128, :], stg[:])


def _p2_load_et(nc, const, et_d):
    et_tiles = []
    for i, (off, cw) in enumerate(_p2_chunks()):
        a = const.tile([H, cw], BF16, tag=f"et{i}")
        nc.sync.dma_start(a[:], et_d[:, off:off + cw])
        et_tiles.append(a)
    return et_tiles


# ---------------------------------------------------------------------------
# program builders
# ---------------------------------------------------------------------------

def _build_phase1(nmax, windows, swin):
    nc = bacc.Bacc("TRN2")
    d = _p1_declare(nc, nmax)
    d["s_h"] = nc.dram_tensor("s_h", [SEG_C, H], F32, kind="ExternalOutput")
    with tile.TileContext(nc) as tc:
        with (
            tc.tile_pool(name="const", bufs=1) as const,
            tc.tile_pool(name="work", bufs=3) as work,
            tc.tile_pool(name="wk2", bufs=3) as wk2,
            tc.tile_pool(name="big", bufs=7, space="PSUM") as big,
            tc.tile_pool(name="sgp", bufs=1, space="PSUM") as sgp,
        ):
            shs = _p1_body(nc, tc, d, nmax, windows, swin,
                           (const, work, wk2, big, sgp))
            for t in range(2):
                nc.sync.dma_start(d["s_h"][t * 128:(t + 1) * 128, :],
                                  shs[:, t, :])
    nc.compile()
    return nc


def _build_phase2():
    nc = bacc.Bacc("TRN2")
    shT_d = nc.dram_tensor("shT", [H, B_SEG], BF16, kind="ExternalInput")
    et_d = nc.dram_tensor("ET", [H, VSHARD], BF16, kind="ExternalInput")
    z_d = nc.dram_tensor("z", [B_SEG, VSHARD], BF16, kind="ExternalOutput")
    with tile.TileContext(nc) as tc:
        with (
            tc.tile_pool(name="const", bufs=1) as const,
            tc.tile_pool(name="stage", bufs=3) as stage,
            tc.tile_pool(name="big", bufs=4, space="PSUM") as big,
        ):
            shT = const.tile([H, B_SEG], BF16)
            nc.sync.dma_start(shT[:], shT_d[:, :])
            et_tiles = _p2_load_et(nc, const, et_d)
            _p2_body(nc, tc, shT, et_tiles, z_d, big, stage, tilew=1024)
    nc.compile()
    return nc


def _build_merged(nmax, windows, swin):
    nc = bacc.Bacc("TRN2", num_devices=8)
    d = _p1_declare(nc, nmax)
    et_d = nc.dram_tensor("ET", [H, VSHARD], BF16, kind="ExternalInput")
    z_d = nc.dram_tensor("z", [B_SEG, VSHARD], BF16, kind="ExternalOutput")
    cc_in = nc.dram_tensor("cc_in", [SEG_C, H], F32)
    cc_out = nc.dram_tensor("cc_out", [B_SEG, H], F32, addr_space="Shared")
    with tile.TileContext(nc) as tc:
        with (
            tc.tile_pool(name="const", bufs=1) as const,
            tc.tile_pool(name="work", bufs=3) as work,
            tc.tile_pool(name="wk2", bufs=3) as wk2,
            tc.tile_pool(name="stage", bufs=3) as stage,
            tc.tile_pool(name="big", bufs=7, space="PSUM") as big,
            tc.tile_pool(name="sgp", bufs=1, space="PSUM") as sgp,
        ):
            ident = const.tile([128, 128], F32)
            make_identity(nc, ident[:])
            et_tiles = _p2_load_et(nc, const, et_d)
            shs = _p1_body(nc, tc, d, nmax, windows, swin,
                           (const, work, wk2, big, sgp))
            for t in range(2):
                nc.sync.dma_start(cc_in[t * 128:(t + 1) * 128, :],
                                  shs[:, t, :])

            # ---- all-gather s_h across the 8 cores ----
            nc.gpsimd.collective_compute(
                "AllGather", mybir.AluOpType.bypass,
                replica_groups=[list(range(8))],
                ins=[cc_in[:, :]], outs=[cc_out[:, :]])

            # ---- shT = gathered s_h transposed, cast bf16 ----
            shT = const.tile([H, B_SEG], BF16)
            for a in range(16):
                gt = wk2.tile([128, 128], F32, tag="gt")
                nc.sync.dma_start(gt[:], cc_out[a * 128:(a + 1) * 128, :])
                gp = big.tile([128, 128], F32, tag="bigp")
                nc.tensor.transpose(gp[:], gt[:], ident[:])
                if a % 2 == 0:
                    nc.vector.tensor_copy(shT[:, a * 128:(a + 1) * 128], gp[:])
                else:
                    nc.scalar.copy(shT[:, a * 128:(a + 1) * 128], gp[:])

            _p2_body(nc, tc, shT, et_tiles, z_d, big, stage)
    nc.compile()
    return nc


# ---------------------------------------------------------------------------
# fallback (windowless) phase 1 — full-width masks, f32, per-chunk gathers
# ---------------------------------------------------------------------------

def _bc(ap, ins_axis, n):
    l = list(ap.ap)
    l.insert(ins_axis, [0, n])
    return bass.AP(tensor=ap.tensor, offset=ap.offset, ap=l)


def _build_phase1_fallback(nmax):
    nt = nmax // 128
    ng = nmax // 512
    nc = bacc.Bacc("TRN2")
    d = {}
    d["x"] = nc.dram_tensor("x", [nmax, H], F32, kind="ExternalInput")
    d["xT"] = nc.dram_tensor("xT", [H, nmax], F32, kind="ExternalInput")
    d["blf"] = nc.dram_tensor("blf", [128, nt], F32, kind="ExternalInput")
    d["bli"] = nc.dram_tensor("bli", [128, nt], I32, kind="ExternalInput")
    d["lastloc"] = nc.dram_tensor("lastloc", [128, 2], I32, kind="ExternalInput")
    d["W1T"] = nc.dram_tensor("W1T", [H, H], F32, kind="ExternalInput")
    d["W2T"] = nc.dram_tensor("W2T", [H, H], F32, kind="ExternalInput")
    d["W3aT"] = nc.dram_tensor("W3aT", [H, H], F32, kind="ExternalInput")
    d["W3bT"] = nc.dram_tensor("W3bT", [H, H], F32, kind="ExternalInput")
    d["b12"] = nc.dram_tensor("b12", [1, H], F32, kind="ExternalInput")
    d["w3brow"] = nc.dram_tensor("w3brow", [1, H], F32, kind="ExternalInput")
    d["qrow"] = nc.dram_tensor("qrow", [1, H], F32, kind="ExternalInput")
    d["qb"] = nc.dram_tensor("qb", [1, 1], F32, kind="ExternalInput")
    d["s_h"] = nc.dram_tensor("s_h", [SEG_C, H], F32, kind="ExternalOutput")
    w1b2_d = nc.dram_tensor("w1b2_scratch", [SEG_C, H], F32)

    with tile.TileContext(nc) as tc:
        with (
            tc.tile_pool(name="const", bufs=1) as const,
            tc.tile_pool(name="xs", bufs=3) as xs,
            tc.tile_pool(name="work", bufs=3) as work,
            tc.tile_pool(name="ps", bufs=2, space="PSUM") as ps,
            tc.tile_pool(name="psw", bufs=3, space="PSUM") as psw,
            tc.tile_pool(name="sgp", bufs=1, space="PSUM") as sgp,
        ):
            ident = const.tile([128, 128], F32)
            make_identity(nc, ident[:])
            iota_i = const.tile([128, SEG_C], I32)
            nc.gpsimd.iota(iota_i[:], pattern=[[1, SEG_C]], base=0,
                           channel_multiplier=0)
            iota_f = const.tile([128, SEG_C], F32)
            nc.vector.tensor_copy(iota_f[:], iota_i[:])
            ones1 = const.tile([1, 128], F32)
            nc.vector.memset(ones1[:], 1.0)
            w1t = const.tile([H, H], F32)
            nc.sync.dma_start(w1t, d["W1T"][:, :])
            w2t = const.tile([H, H], F32)
            nc.sync.dma_start(w2t, d["W2T"][:, :])
            w3at = const.tile([H, H], F32)
            nc.sync.dma_start(w3at, d["W3aT"][:, :])
            w3bt = const.tile([H, H], F32)
            nc.sync.dma_start(w3bt, d["W3bT"][:, :])
            b12 = const.tile([1, H], F32)
            nc.sync.dma_start(b12, d["b12"][:, :])
            w3brow = const.tile([1, H], F32)
            nc.sync.dma_start(w3brow, d["w3brow"][:, :])
            qrow = const.tile([1, H], F32)
            nc.sync.dma_start(qrow[:], d["qrow"][:, :])
            qb = const.tile([128, 1], F32)
            nc.sync.dma_start(qb, d["qb"][:, :].partition_broadcast(128))
            blf = const.tile([128, nt], F32)
            nc.sync.dma_start(blf[:], d["blf"][:, :])
            bli = const.tile([128, nt], I32)
            nc.sync.dma_start(bli[:], d["bli"][:, :])
            lastloc = const.tile([128, 2], I32)
            nc.sync.dma_start(lastloc[:], d["lastloc"][:, :])

            qps = ps.tile([128, 128], F32, tag="mm")
            nc.tensor.matmul(qps[:], ones1[:], qrow[:], start=True, stop=True)
            q_bcast = const.tile([128, 128], F32)
            nc.vector.tensor_copy(q_bcast[:], qps[:])

            vn = const.tile([128, 2, H], F32)
            vnT = const.tile([H, SEG_C], F32)
            w1b2 = const.tile([128, 2, H], F32)
            for t in range(2):
                nc.gpsimd.indirect_dma_start(
                    out=vn[:, t, :], out_offset=None, in_=d["x"][:, :],
                    in_offset=bass.IndirectOffsetOnAxis(
                        ap=lastloc[:, t:t + 1], axis=0))
                tp = ps.tile([128, 128], F32, tag="mm")
                nc.tensor.transpose(tp[:], vn[:, t, :], ident[:])
                nc.vector.tensor_copy(vnT[:, t * 128:(t + 1) * 128], tp[:])
                pw = ps.tile([128, 128], F32, tag="mm")
                nc.tensor.matmul(pw[:], ones1[:], b12, start=True, stop=False)
                nc.tensor.matmul(pw[:], vnT[:, t * 128:(t + 1) * 128], w1t,
                                 start=False, stop=True)
                nc.vector.tensor_copy(w1b2[:, t, :], pw[:])
                nc.sync.dma_start(w1b2_d[t * 128:(t + 1) * 128, :], w1b2[:, t, :])

            sg_ps = sgp.tile([128, SEG_C], F32)
            for g in range(ng):
                x_sb = xs.tile([128, 4, H], F32)
                nc.sync.dma_start(
                    x_sb[:],
                    d["x"][g * 512:(g + 1) * 512, :].rearrange(
                        "(c p) h -> p c h", p=128))
                xT_sb = xs.tile([H, 512], F32)
                nc.sync.dma_start(xT_sb[:], d["xT"][:, g * 512:(g + 1) * 512])

                p1g = psw.tile([128, 512], F32, tag="p1")
                for c in range(4):
                    nc.tensor.matmul(p1g[:, c * 128:(c + 1) * 128],
                                     xT_sb[:, c * 128:(c + 1) * 128],
                                     w2t, start=True, stop=True)
                hpre = work.tile([128, 4, H], F32)
                hpre_flat = hpre[:].rearrange("p a b -> p (a b)")
                nc.scalar.copy(hpre_flat, p1g[:])
                for c in range(4):
                    nc.gpsimd.indirect_dma_start(
                        out=hpre[:, c, :], out_offset=None, in_=w1b2_d[:, :],
                        in_offset=bass.IndirectOffsetOnAxis(
                            ap=bli[:, 4 * g + c:4 * g + c + 1], axis=0),
                        compute_op=mybir.AluOpType.add)
                hsb = work.tile([128, 4, H], F32)
                nc.scalar.activation(hsb[:].rearrange("p a b -> p (a b)"),
                                     hpre_flat,
                                     mybir.ActivationFunctionType.Sigmoid)
                hq = work.tile([128, 4, H], F32)
                nc.vector.tensor_tensor(hq[:], hsb[:], _bc(q_bcast[:], 1, 4),
                                        op=mybir.AluOpType.mult)
                araw = work.tile([128, 4], F32)
                nc.vector.reduce_sum(araw[:], hq[:], axis=mybir.AxisListType.X)
                alpha = work.tile([128, 4], F32)
                nc.vector.tensor_tensor(alpha[:], araw[:],
                                        qb.to_broadcast([128, 4]),
                                        op=mybir.AluOpType.add)
                mask = work.tile([128, 4, SEG_C], F32, tag="ma")
                for c in range(4):
                    n = g * 4 + c
                    nc.vector.tensor_scalar(
                        mask[:, c, :], iota_f[:],
                        blf[:, n:n + 1], alpha[:, c:c + 1],
                        mybir.AluOpType.is_equal, mybir.AluOpType.mult)
                    nc.tensor.matmul(sg_ps[:], x_sb[:, c, :], mask[:, c, :],
                                     start=(n == 0), stop=(n == nt - 1))

            sgT = const.tile([H, SEG_C], F32)
            nc.vector.tensor_copy(sgT[:], sg_ps[:])
            shs = const.tile([128, 2, H], F32)
            for t in range(2):
                psh = ps.tile([128, 128], F32, tag="mm")
                nc.tensor.matmul(psh[:], ones1[:], w3brow, start=True,
                                 stop=False)
                nc.tensor.matmul(psh[:], vnT[:, t * 128:(t + 1) * 128],
                                 w3at, start=False, stop=False)
                nc.tensor.matmul(psh[:], sgT[:, t * 128:(t + 1) * 128],
                                 w3bt, start=False, stop=True)
                nc.vector.tensor_copy(shs[:, t, :], psh[:])
                nc.sync.dma_start(d["s_h"][t * 128:(t + 1) * 128, :],
                                  shs[:, t, :])
    nc.compile()
    return nc


# ---------------------------------------------------------------------------
# host-side prep
# ---------------------------------------------------------------------------

def _nmax_for(batch):
    starts = np.searchsorted(batch, np.arange(0, B_SEG + 1, SEG_C))
    counts = np.diff(starts)
    return int(-(-counts.max() // 512) * 512)


def _prep(inputs):
    """Shard inputs, derive index tensors and one-hot masks from `batch`."""
    batch = np.asarray(inputs["batch"]).astype(np.int64)
    x = np.ascontiguousarray(np.asarray(inputs["session_embedding"], np.float32))
    emb = np.ascontiguousarray(np.asarray(inputs["emb_weight"], np.float32))

    starts = np.searchsorted(batch, np.arange(0, B_SEG + 1, SEG_C))
    nmax = _nmax_for(batch)
    nt = nmax // 128

    last_idx = np.searchsorted(batch, np.arange(B_SEG) + 1) - 1  # [B]

    w1t = np.ascontiguousarray(np.asarray(inputs["W1_w"], np.float32).T)
    w2t = np.ascontiguousarray(np.asarray(inputs["W2_w"], np.float32).T)
    w3 = np.asarray(inputs["W3_w"], np.float32)
    w3at = np.ascontiguousarray(w3[:, :H].T)
    w3bt = np.ascontiguousarray(w3[:, H:].T)
    b12 = (np.asarray(inputs["W1_b"], np.float32)
           + np.asarray(inputs["W2_b"], np.float32)).reshape(1, H)
    w3brow = np.asarray(inputs["W3_b"], np.float32).reshape(1, H)
    qrow = np.asarray(inputs["q_w"], np.float32).reshape(1, H)
    qb = np.asarray(inputs["q_b"], np.float32).reshape(1, 1)

    pbf = np.zeros((H, 5 * H), np.float32)
    pbf[:, 0 * H:1 * H] = w2t
    pbf[:, 1 * H:2 * H] = w1t
    pbf[:, 2 * H:3 * H] = w3at
    pbf[:, 3 * H:4 * H] = w3bt
    pbf[:, 4 * H:4 * H + 1] = qrow.reshape(H, 1)
    pbf = np.ascontiguousarray(pbf).astype(NPBF)
    pf32 = np.zeros((1, 3 * H), np.float32)
    pf32[:, 0 * H:1 * H] = b12
    pf32[:, 1 * H:2 * H] = w3brow
    pf32[:, 2 * H] = qb[0, 0]
    pf32 = np.ascontiguousarray(pf32)

    windows = affine_windows(nmax, MW, grid=MGRID)
    swin = affine_windows(nmax, SW)
    warr = np.repeat(np.asarray(windows, np.int64), 128)
    sarr = np.repeat(np.asarray(swin, np.int64), 128)
    rngMW = np.arange(MW, dtype=np.int64)
    rngSW = np.arange(SW, dtype=np.int64)

    in1, in2, fb1 = [], [], []
    ok = True
    for c in range(NCORES):
        st, en = int(starts[c]), int(starts[c + 1])
        cnt = en - st
        xc = np.zeros((nmax, H), np.float32)
        xc[:cnt] = x[st:en]
        xb = xc.astype(NPBF)
        blc = np.full(nmax, SEG_C - 1, np.int64)
        blc[:cnt] = batch[st:en] - c * SEG_C
        lastl = (last_idx[c * SEG_C:(c + 1) * SEG_C] - st).astype(np.int32)

        rel = blc - warr
        srel = blc - sarr
        ok = ok and bool(((rel >= 0) & (rel < MW)).all()
                         and ((srel >= 0) & (srel < SW)).all())

        segmask = (rel[None, :] == rngMW[:, None]).astype(NPF8)
        nodemask = np.ascontiguousarray(
            (srel.reshape(nt, 128)[:, :, None] == rngSW[None, None, :])
            .transpose(1, 0, 2).reshape(128, nt * SW).astype(NPF8))
        xph = np.ascontiguousarray(
            xb.reshape(nt, 128, H).transpose(1, 0, 2)).reshape(128, nt * H)

        in1.append({
            "x": xb,
            "xT": np.ascontiguousarray(xb.T),
            "xph": xph,
            "segmask": np.ascontiguousarray(segmask),
            "nodemask": nodemask,
            "lastloc": np.ascontiguousarray(lastl.reshape(2, 128).T),
            "pbf": pbf, "pf32": pf32,
        })
        fb1.append({
            "xc": xc, "blc": blc, "lastl": lastl,
            "W1T": w1t, "W2T": w2t, "W3aT": w3at, "W3bT": w3bt,
            "b12": b12, "w3brow": w3brow, "qrow": qrow, "qb": qb,
        })
        v0 = 1 + c * VSHARD
        v1 = min(v0 + VSHARD, VOCAB)
        etc = np.zeros((VSHARD, H), np.float32)
        etc[:v1 - v0] = emb[v0:v1]
        in2.append({"ET": np.ascontiguousarray(etc.T).astype(NPBF)})
    return in1, in2, fb1, nmax, ok


_CACHE = {}


def _get_programs(nmax, windowed=True):
    key = (nmax, windowed)
    if key not in _CACHE:
        if windowed:
            nc1 = _build_phase1(nmax, affine_windows(nmax, MW, grid=MGRID),
                                affine_windows(nmax, SW))
        else:
            nc1 = _build_phase1_fallback(nmax)
        _CACHE[key] = (nc1, _build_phase2())
    return _CACHE[key]


def _get_merged(nmax):
    key = ("merged", nmax)
    if key not in _CACHE:
        _CACHE[key] = _build_merged(nmax, affine_windows(nmax, MW, grid=MGRID),
                                    affine_windows(nmax, SW))
    return _CACHE[key]


def _assemble_z(res):
    z = np.concatenate(
        [np.asarray(res.results[c]["z"]).astype(np.float32)
         for c in range(NCORES)], axis=1)
    return np.ascontiguousarray(z[:, :VOCAB - 1])


def kernel(**inputs) -> np.ndarray:
    in1, in2, fb1, nmax, windowed = _prep(inputs)

    if windowed:
        # single launch: phase1 + on-device AllGather of s_h + phase2
        nc = _get_merged(nmax)
        ins = []
        for c in range(NCORES):
            m = dict(in1[c])
            m.update(in2[c])
            ins.append(m)
        res = bass_utils.run_bass_kernel_spmd(nc, ins,
                                              core_ids=list(range(NCORES)))
        return _assemble_z(res)

    # fallback: two launches with host gather of s_h
    nt = nmax // 128
    fbin = []
    for m in fb1:
        xc, blc, lastl = m.pop("xc"), m.pop("blc"), m.pop("lastl")
        m["x"] = xc
        m["xT"] = np.ascontiguousarray(xc.T)
        m["blf"] = np.ascontiguousarray(blc.reshape(nt, 128).T.astype(np.float32))
        m["bli"] = np.ascontiguousarray(blc.reshape(nt, 128).T.astype(np.int32))
        m["lastloc"] = np.ascontiguousarray(lastl.reshape(2, 128).T)
        fbin.append(m)
    nc1, nc2 = _get_programs(nmax, windowed=False)
    res1 = bass_utils.run_bass_kernel_spmd(nc1, fbin, core_ids=list(range(NCORES)))
    sh = np.concatenate([np.asarray(res1.results[c]["s_h"])
                         for c in range(NCORES)], axis=0)
    shT = np.ascontiguousarray(sh.T).astype(NPBF)  # [H, B_SEG]
    for m in in2:
        m["shT"] = shT
    res2 = bass_utils.run_bass_kernel_spmd(nc2, in2, core_ids=list(range(NCORES)))
    return _assemble_z(res2)

